# revision 5
# baseline (speedup 1.0000x reference)
"""Trainium2 Bass kernel for nn_MaxCDFdp_multiclass.

Computes max over (class, probe) of |ECDF0 - ECDF1| where the ECDFs are
sigmoid-smoothed empirical CDFs of y_pred per class for the two groups
defined by s in {0,1}.

v5: rank-block sample compression + PE-generated window args.

Compression: the sigmoid-smoothed CDF is a quadrature over samples; K
consecutive order statistics whose span is <= TAU/TEMP can be replaced
by their mean with weight K (second-order quadrature error, ~2e-6 on
delta vs a 2.2e-4 budget). One rank-block schedule per group shared by
all 20 classes keeps the weight per pseudo-sample row uniform across
classes, so it folds into the reduction matmul's stationary mask.
25000 samples/group compress to ~500 weighted rows; all 8 cores hold
~991 rows -> ONE [128, C*W] tile per core.

Device per core (all operands fp16, PSUM f32):
  DMA in : a = [aug | eg] [C+1, W*C + 128], mk [128, 2]
  PE     : diff[128, C*W] = aug[C+1,128].T @ eg[C+1,C*W]  (affine args)
  ACT    : sig = sigmoid(TEMP * diff)  PSUM -> SBUF fp16
  PE     : acc[2, C*W] = mk[128,2].T @ sig  (weighted group sums)
  DVE/ACT: acc PSUM -> SBUF
  DMA out: [2, C*W] f32
Host: relocate each core's [2, C, W] window into [2, C, P] at its
per-class base B (+ weighted saturated tail), divide by group counts,
abs, max.
"""

import os
from contextlib import ExitStack

import numpy as np

import concourse.bass as bass
import concourse.bacc as bacc
import concourse.tile as tile
from concourse import mybir
from concourse.bass_utils import run_bass_kernel_spmd

N, C, P = 50000, 20, 100
TEMP = 10.0
NCORES = 8
PART = 128
W = 56                  # probe-window width per core-tile
CW = C * W              # 1120
K1 = C + 1              # contraction dim of the affine matmul
TAU = 0.2               # rank-block span limit in sigmoid-arg units
MARGIN = 1.15           # |grid - y| saturation cutoff (11.5 in arg units)

_F16 = mybir.dt.float16
_F32 = mybir.dt.float32

# matmul free-dim chunks within single PSUM banks (512 f32/bank)
_CHUNKS = [(0, 512), (512, 1024), (1024, CW)]

_CACHED = {}


def _build_bass():
    nc = bacc.Bacc(None, target_bir_lowering=False)
    a_d = nc.dram_tensor("a", [K1, CW + PART], _F16, kind="ExternalInput")
    m_d = nc.dram_tensor("m", [PART, 2], _F16, kind="ExternalInput")
    o_d = nc.dram_tensor("o", [2, CW], _F32, kind="ExternalOutput")

    with ExitStack() as ctx:
        tc = ctx.enter_context(tile.TileContext(nc))
        constp = ctx.enter_context(tc.tile_pool(name="const", bufs=1))
        psump = ctx.enter_context(
            tc.tile_pool(name="psum", bufs=1, space=bass.MemorySpace.PSUM)
        )

        aug = constp.tile([K1, CW + PART], _F16)
        nc.sync.dma_start(aug[:], a_d[:])
        mk = constp.tile([PART, 2], _F16)
        nc.sync.dma_start(mk[:], m_d[:])
        lhs = aug[:, CW : CW + PART]     # [K1, 128] per-row A' + ones row
        eg = aug[:, 0:CW]                # [K1, CW] selector + Dw rows

        sig = constp.tile([PART, CW], _F16)
        out_sb = constp.tile([2, CW], _F32)

        # p-state warmup: keep the PE busy while the input DMA is in
        # flight so the real matmuls run at full clock
        scratch = constp.tile([PART, 512], _F16)
        nc.gpsimd.memset(scratch[:], 0.0)
        wps = psump.tile([2, 512], _F32, tag="warm")
        for _ in range(8):
            nc.tensor.matmul(
                wps[:], scratch[:, 0:2], scratch[:], start=True, stop=True
            )

        for i, (c0, c1) in enumerate(_CHUNKS):
            dps = psump.tile([PART, c1 - c0], _F32, tag=f"d{i}")
            nc.tensor.matmul(dps[:], lhs, eg[:, c0:c1], start=True, stop=True)
            nc.scalar.activation(
                sig[:, c0:c1], dps[:],
                mybir.ActivationFunctionType.Sigmoid, scale=TEMP,
            )
            acc = psump.tile([2, c1 - c0], _F32, tag=f"a{i}")
            nc.tensor.matmul(acc[:], mk[:], sig[:, c0:c1], start=True, stop=True)
            eng = nc.vector.tensor_copy if i % 2 == 0 else nc.scalar.copy
            eng(out_sb[:, c0:c1], acc[:])

        nc.sync.dma_start(o_d[:], out_sb[:])

    nc.finalize()
    return nc


def _get_nc():
    if "nc" not in _CACHED:
        _CACHED["nc"] = _build_bass()
    return _CACHED["nc"]


def _rank_merge(srt, tau):
    """Greedy shared-rank blocks: max K with max_c span <= tau."""
    n = srt.shape[0]
    starts, sizes = [], []
    r = 0
    while r < n:
        lo, hi = 1, n - r
        while lo < hi:
            mid = (lo + hi + 1) // 2
            if (srt[r + mid - 1] - srt[r]).max() <= tau:
                lo = mid
            else:
                hi = mid - 1
        starts.append(r)
        sizes.append(lo)
        r += lo
    vals = np.stack(
        [srt[a : a + k].mean(0, dtype=np.float64) for a, k in zip(starts, sizes)]
    )
    return vals.astype(np.float32), np.asarray(sizes, np.float64)


# test.py reads these after calling kernel()
LAST_RESULTS = None
LAST_DELTA = None


def kernel(y_pred: np.ndarray, s: np.ndarray) -> np.ndarray:
    global LAST_RESULTS, LAST_DELTA
    y = np.ascontiguousarray(np.asarray(y_pred), dtype=np.float32)
    s_np = np.asarray(s)
    assert y.shape == (N, C)

    mn = y.min(axis=0)
    mx = y.max(axis=0)
    step = (mx.astype(np.float64) - mn) / (P - 1)

    srt0 = np.sort(y[s_np == 0], axis=0)
    srt1 = np.sort(y[s_np == 1], axis=0)
    n0, n1 = srt0.shape[0], srt1.shape[0]

    v0, w0 = _rank_merge(srt0, TAU / TEMP)
    v1, w1 = _rank_merge(srt1, TAU / TEMP)

    jj = np.arange(W, dtype=np.float64)

    in_maps = []
    b_tabs = []
    core_meta = []
    for r in range(NCORES):
        vs, ws, gs = [], [], []
        for gi, (v, w) in enumerate(((v0, w0), (v1, w1))):
            idx = np.array_split(np.arange(len(w)), NCORES)[r]
            vs.append(v[idx])
            ws.append(w[idx])
            gs.append(np.full(len(idx), gi))
        vals = np.concatenate(vs)          # [cnt, C]
        wts = np.concatenate(ws)           # [cnt]
        grp = np.concatenate(gs)           # [cnt]
        cnt = len(wts)
        assert cnt <= PART, cnt

        ymax_t = vals.max(0).astype(np.float64)
        ymin_t = vals.min(0).astype(np.float64)
        needW = ((ymax_t - ymin_t + 2 * MARGIN) / step + 2).max()
        assert needW <= W, (needW, W)
        B = np.ceil((ymax_t + MARGIN - mn) / step).astype(np.int64) - W + 1
        B = np.clip(B, 0, P - W)
        base = mn.astype(np.float64) + step * B            # [C]

        # centered window args: diff = A'[m,c] + Dw[c,w]
        A = np.zeros((PART, C), np.float64)
        A[:cnt] = base[None, :] + step * (W / 2) - vals
        A[cnt:] = A[cnt - 1]                                # benign pad
        Dw = step[:, None] * (jj[None, :] - W / 2)          # [C, W]

        a = np.zeros((K1, CW + PART), np.float16)
        a[0:C, CW : CW + PART] = A.T.astype(np.float16)     # aug rows: A'
        a[C, CW : CW + PART] = 1.0                          # ones row
        for c in range(C):
            a[c, c * W : (c + 1) * W] = 1.0                 # selector
        a[C, 0:CW] = Dw.reshape(CW).astype(np.float16)      # Dw row
        mask = np.zeros((PART, 2), np.float16)
        mask[np.arange(cnt), grp] = wts

        in_maps.append({"a": a, "m": mask})
        b_tabs.append(B)
        core_meta.append(mask.sum(0).astype(np.float64))    # [2] group weight

    nc = _get_nc()
    res = run_bass_kernel_spmd(
        nc,
        in_maps,
        core_ids=list(range(NCORES)),
        trace=bool(int(os.environ.get("BASS_KERNEL_TRACE", "0"))),
    )
    LAST_RESULTS = res

    full = np.zeros((2, C, P + W), np.float64)
    for r in range(NCORES):
        o = res.results[r]["o"].astype(np.float64)          # [2, CW]
        acc = o.reshape(2, C, W)
        B = b_tabs[r]
        gw = core_meta[r]
        for c in range(C):
            b = B[c]
            full[:, c, b : b + W] += acc[:, c, :]
            full[0, c, b + W :] += gw[0]
            full[1, c, b + W :] += gw[1]
    fullP = full[:, :, :P]
    delta = np.abs(fullP[0] / n0 - fullP[1] / n1)
    LAST_DELTA = delta
    return np.array(delta.max(), dtype=np.float32)


# revision 6
# speedup vs baseline: 1.0727x; 1.0727x over previous
"""Trainium2 Bass kernel for nn_MaxCDFdp_multiclass.

Computes max over (class, probe) of |ECDF0 - ECDF1| where the ECDFs are
sigmoid-smoothed empirical CDFs of y_pred per class for the two groups
defined by s in {0,1}.

v5: rank-block sample compression + PE-generated window args.

Compression: the sigmoid-smoothed CDF is a quadrature over samples; K
consecutive order statistics whose span is <= TAU/TEMP can be replaced
by their mean with weight K (second-order quadrature error, ~2e-6 on
delta vs a 2.2e-4 budget). One rank-block schedule per group shared by
all 20 classes keeps the weight per pseudo-sample row uniform across
classes, so it folds into the reduction matmul's stationary mask.
25000 samples/group compress to ~500 weighted rows; all 8 cores hold
~991 rows -> ONE [128, C*W] tile per core.

Device per core (all operands fp16, PSUM f32):
  DMA in : a = [aug | eg] [C+1, W*C + 128], mk [128, 2]
  PE     : diff[128, C*W] = aug[C+1,128].T @ eg[C+1,C*W]  (affine args)
  ACT    : sig = sigmoid(TEMP * diff)  PSUM -> SBUF fp16
  PE     : acc[2, C*W] = mk[128,2].T @ sig  (weighted group sums)
  DVE/ACT: acc PSUM -> SBUF
  DMA out: [2, C*W] f32
Host: relocate each core's [2, C, W] window into [2, C, P] at its
per-class base B (+ weighted saturated tail), divide by group counts,
abs, max.
"""

import os
from contextlib import ExitStack

import numpy as np

import concourse.bass as bass
import concourse.bacc as bacc
import concourse.tile as tile
from concourse import mybir
from concourse.bass_utils import run_bass_kernel_spmd

N, C, P = 50000, 20, 100
TEMP = 10.0
NCORES = 8
PART = 128
W = 56                  # probe-window width per core-tile
CW = C * W              # 1120
K1 = C + 1              # contraction dim of the affine matmul
TAU = 0.2               # rank-block span limit in sigmoid-arg units
MARGIN = 1.15           # |grid - y| saturation cutoff (11.5 in arg units)

_F16 = mybir.dt.float16
_F32 = mybir.dt.float32

# matmul free-dim chunks within single PSUM banks (512 f32/bank)
_CHUNKS = [(0, 512), (512, 1024), (1024, CW)]

_CACHED = {}


def _build_bass():
    nc = bacc.Bacc(None, target_bir_lowering=False)
    a_d = nc.dram_tensor("a", [K1, CW + PART], _F16, kind="ExternalInput")
    m_d = nc.dram_tensor("m", [PART, 2], _F16, kind="ExternalInput")
    o_d = nc.dram_tensor("o", [2, CW], _F32, kind="ExternalOutput")

    with ExitStack() as ctx:
        tc = ctx.enter_context(tile.TileContext(nc))
        constp = ctx.enter_context(tc.tile_pool(name="const", bufs=1))
        psump = ctx.enter_context(
            tc.tile_pool(name="psum", bufs=1, space=bass.MemorySpace.PSUM)
        )

        aug = constp.tile([K1, CW + PART], _F16)
        nc.sync.dma_start(aug[:], a_d[:])
        mk = constp.tile([PART, 2], _F16)
        nc.sync.dma_start(mk[:], m_d[:])
        lhs = aug[:, CW : CW + PART]     # [K1, 128] per-row A' + ones row
        eg = aug[:, 0:CW]                # [K1, CW] selector + Dw rows

        sig = constp.tile([PART, CW], _F16)
        out_sb = constp.tile([2, CW], _F32)

        for i, (c0, c1) in enumerate(_CHUNKS):
            dps = psump.tile([PART, c1 - c0], _F32, tag=f"d{i}")
            nc.tensor.matmul(dps[:], lhs, eg[:, c0:c1], start=True, stop=True)
            nc.scalar.activation(
                sig[:, c0:c1], dps[:],
                mybir.ActivationFunctionType.Sigmoid, scale=TEMP,
            )
            acc = psump.tile([2, c1 - c0], _F32, tag=f"a{i}")
            nc.tensor.matmul(acc[:], mk[:], sig[:, c0:c1], start=True, stop=True)
            if i == 1:
                # split the big mid-chunk drain across two engines so the
                # final output DMA isn't gated on one long copy
                h = (c1 - c0) // 2
                nc.vector.tensor_copy(out_sb[:, c0 : c0 + h], acc[:, 0:h])
                nc.scalar.copy(out_sb[:, c0 + h : c1], acc[:, h:])
            else:
                nc.vector.tensor_copy(out_sb[:, c0:c1], acc[:])

        nc.sync.dma_start(o_d[:], out_sb[:])

    nc.finalize()
    return nc


def _get_nc():
    if "nc" not in _CACHED:
        _CACHED["nc"] = _build_bass()
    return _CACHED["nc"]


def _rank_merge(srt, tau):
    """Greedy shared-rank blocks: max K with max_c span <= tau."""
    n = srt.shape[0]
    starts, sizes = [], []
    r = 0
    while r < n:
        lo, hi = 1, n - r
        while lo < hi:
            mid = (lo + hi + 1) // 2
            if (srt[r + mid - 1] - srt[r]).max() <= tau:
                lo = mid
            else:
                hi = mid - 1
        starts.append(r)
        sizes.append(lo)
        r += lo
    vals = np.stack(
        [srt[a : a + k].mean(0, dtype=np.float64) for a, k in zip(starts, sizes)]
    )
    return vals.astype(np.float32), np.asarray(sizes, np.float64)


# test.py reads these after calling kernel()
LAST_RESULTS = None
LAST_DELTA = None


def kernel(y_pred: np.ndarray, s: np.ndarray) -> np.ndarray:
    global LAST_RESULTS, LAST_DELTA
    y = np.ascontiguousarray(np.asarray(y_pred), dtype=np.float32)
    s_np = np.asarray(s)
    assert y.shape == (N, C)

    mn = y.min(axis=0)
    mx = y.max(axis=0)
    step = (mx.astype(np.float64) - mn) / (P - 1)

    srt0 = np.sort(y[s_np == 0], axis=0)
    srt1 = np.sort(y[s_np == 1], axis=0)
    n0, n1 = srt0.shape[0], srt1.shape[0]

    v0, w0 = _rank_merge(srt0, TAU / TEMP)
    v1, w1 = _rank_merge(srt1, TAU / TEMP)

    jj = np.arange(W, dtype=np.float64)

    in_maps = []
    b_tabs = []
    core_meta = []
    for r in range(NCORES):
        vs, ws, gs = [], [], []
        for gi, (v, w) in enumerate(((v0, w0), (v1, w1))):
            idx = np.array_split(np.arange(len(w)), NCORES)[r]
            vs.append(v[idx])
            ws.append(w[idx])
            gs.append(np.full(len(idx), gi))
        vals = np.concatenate(vs)          # [cnt, C]
        wts = np.concatenate(ws)           # [cnt]
        grp = np.concatenate(gs)           # [cnt]
        cnt = len(wts)
        assert cnt <= PART, cnt

        ymax_t = vals.max(0).astype(np.float64)
        ymin_t = vals.min(0).astype(np.float64)
        needW = ((ymax_t - ymin_t + 2 * MARGIN) / step + 2).max()
        assert needW <= W, (needW, W)
        B = np.ceil((ymax_t + MARGIN - mn) / step).astype(np.int64) - W + 1
        B = np.clip(B, 0, P - W)
        base = mn.astype(np.float64) + step * B            # [C]

        # centered window args: diff = A'[m,c] + Dw[c,w]
        A = np.zeros((PART, C), np.float64)
        A[:cnt] = base[None, :] + step * (W / 2) - vals
        A[cnt:] = A[cnt - 1]                                # benign pad
        Dw = step[:, None] * (jj[None, :] - W / 2)          # [C, W]

        a = np.zeros((K1, CW + PART), np.float16)
        a[0:C, CW : CW + PART] = A.T.astype(np.float16)     # aug rows: A'
        a[C, CW : CW + PART] = 1.0                          # ones row
        for c in range(C):
            a[c, c * W : (c + 1) * W] = 1.0                 # selector
        a[C, 0:CW] = Dw.reshape(CW).astype(np.float16)      # Dw row
        mask = np.zeros((PART, 2), np.float16)
        mask[np.arange(cnt), grp] = wts

        in_maps.append({"a": a, "m": mask})
        b_tabs.append(B)
        core_meta.append(mask.sum(0).astype(np.float64))    # [2] group weight

    nc = _get_nc()
    res = run_bass_kernel_spmd(
        nc,
        in_maps,
        core_ids=list(range(NCORES)),
        trace=bool(int(os.environ.get("BASS_KERNEL_TRACE", "0"))),
    )
    LAST_RESULTS = res

    full = np.zeros((2, C, P + W), np.float64)
    for r in range(NCORES):
        o = res.results[r]["o"].astype(np.float64)          # [2, CW]
        acc = o.reshape(2, C, W)
        B = b_tabs[r]
        gw = core_meta[r]
        for c in range(C):
            b = B[c]
            full[:, c, b : b + W] += acc[:, c, :]
            full[0, c, b + W :] += gw[0]
            full[1, c, b + W :] += gw[1]
    fullP = full[:, :, :P]
    delta = np.abs(fullP[0] / n0 - fullP[1] / n1)
    LAST_DELTA = delta
    return np.array(delta.max(), dtype=np.float32)


# revision 9
# speedup vs baseline: 1.1090x; 1.0339x over previous
"""Trainium2 Bass kernel for nn_MaxCDFdp_multiclass.

Computes max over (class, probe) of |ECDF0 - ECDF1| where the ECDFs are
sigmoid-smoothed empirical CDFs of y_pred per class for the two groups
defined by s in {0,1}.

v5: rank-block sample compression + PE-generated window args.

Compression: the sigmoid-smoothed CDF is a quadrature over samples; K
consecutive order statistics whose span is <= TAU/TEMP can be replaced
by their mean with weight K (second-order quadrature error, ~2e-6 on
delta vs a 2.2e-4 budget). One rank-block schedule per group shared by
all 20 classes keeps the weight per pseudo-sample row uniform across
classes, so it folds into the reduction matmul's stationary mask.
25000 samples/group compress to ~500 weighted rows; all 8 cores hold
~991 rows -> ONE [128, C*W] tile per core.

Device per core (all operands fp16, PSUM f32):
  DMA in : a = [aug | eg] [C+1, W*C + 128], mk [128, 2]
  PE     : diff[128, C*W] = aug[C+1,128].T @ eg[C+1,C*W]  (affine args)
  ACT    : sig = sigmoid(TEMP * diff)  PSUM -> SBUF fp16
  PE     : acc[2, C*W] = mk[128,2].T @ sig  (weighted group sums)
  DVE/ACT: acc PSUM -> SBUF
  DMA out: [2, C*W] f32
Host: relocate each core's [2, C, W] window into [2, C, P] at its
per-class base B (+ weighted saturated tail), divide by group counts,
abs, max.
"""

import os
from contextlib import ExitStack

import numpy as np

import concourse.bass as bass
import concourse.bacc as bacc
import concourse.tile as tile
from concourse import mybir
from concourse.bass_utils import run_bass_kernel_spmd

N, C, P = 50000, 20, 100
TEMP = 10.0
NCORES = 8
PART = 128
W = 56                  # probe-window width per core-tile
CW = C * W              # 1120
K1 = C + 1              # contraction dim of the affine matmul
TAU = 0.2               # rank-block span limit in sigmoid-arg units
MARGIN = 1.15           # |grid - y| saturation cutoff (11.5 in arg units)

_F16 = mybir.dt.float16
_F32 = mybir.dt.float32

# matmul free-dim chunks within single PSUM banks (512 f32/bank)
_CHUNKS = [(0, 512), (512, 1024), (1024, CW)]

_CACHED = {}


def _build_bass():
    nc = bacc.Bacc(None, target_bir_lowering=False)
    c0_, c1_ = _CHUNKS[0][1], CW - _CHUNKS[0][1]
    a_d = nc.dram_tensor("a", [K1, PART + c0_], _F16, kind="ExternalInput")
    a2_d = nc.dram_tensor("a2", [K1, c1_], _F16, kind="ExternalInput")
    m_d = nc.dram_tensor("m", [PART, 2], _F16, kind="ExternalInput")
    o_d = nc.dram_tensor("o", [2, CW], _F32, kind="ExternalOutput")

    with ExitStack() as ctx:
        tc = ctx.enter_context(tile.TileContext(nc))
        constp = ctx.enter_context(tc.tile_pool(name="const", bufs=1))
        psump = ctx.enter_context(
            tc.tile_pool(name="psum", bufs=1, space=bass.MemorySpace.PSUM)
        )

        augA = constp.tile([K1, PART + c0_], _F16)
        nc.sync.dma_start(augA[:], a_d[:])
        augB = constp.tile([K1, c1_], _F16)
        nc.sync.dma_start(augB[:], a2_d[:])
        mk = constp.tile([PART, 2], _F16)
        nc.sync.dma_start(mk[:], m_d[:])
        lhs = augA[:, 0:PART]            # [K1, 128] per-row A' + ones row

        sig = constp.tile([PART, CW], _F16)
        out_sb = constp.tile([2, CW], _F32)

        for i, (c0, c1) in enumerate(_CHUNKS):
            eg = augA[:, PART + c0 : PART + c1] if i == 0 else \
                augB[:, c0 - c0_ : c1 - c0_]
            dps = psump.tile([PART, c1 - c0], _F32, tag=f"d{i}")
            nc.tensor.matmul(dps[:], lhs, eg, start=True, stop=True)
            nc.scalar.activation(
                sig[:, c0:c1], dps[:],
                mybir.ActivationFunctionType.Sigmoid, scale=TEMP,
            )
            acc = psump.tile([2, c1 - c0], _F32, tag=f"a{i}")
            nc.tensor.matmul(acc[:], mk[:], sig[:, c0:c1], start=True, stop=True)
            eng = nc.scalar.copy if i == 1 else nc.vector.tensor_copy
            eng(out_sb[:, c0:c1], acc[:])

        nc.sync.dma_start(o_d[:], out_sb[:])

    nc.finalize()
    return nc


def _get_nc():
    if "nc" not in _CACHED:
        _CACHED["nc"] = _build_bass()
    return _CACHED["nc"]


def _rank_merge(srt, tau):
    """Greedy shared-rank blocks: max K with max_c span <= tau."""
    n = srt.shape[0]
    starts, sizes = [], []
    r = 0
    while r < n:
        lo, hi = 1, n - r
        while lo < hi:
            mid = (lo + hi + 1) // 2
            if (srt[r + mid - 1] - srt[r]).max() <= tau:
                lo = mid
            else:
                hi = mid - 1
        starts.append(r)
        sizes.append(lo)
        r += lo
    vals = np.stack(
        [srt[a : a + k].mean(0, dtype=np.float64) for a, k in zip(starts, sizes)]
    )
    return vals.astype(np.float32), np.asarray(sizes, np.float64)


# test.py reads these after calling kernel()
LAST_RESULTS = None
LAST_DELTA = None


def kernel(y_pred: np.ndarray, s: np.ndarray) -> np.ndarray:
    global LAST_RESULTS, LAST_DELTA
    y = np.ascontiguousarray(np.asarray(y_pred), dtype=np.float32)
    s_np = np.asarray(s)
    assert y.shape == (N, C)

    mn = y.min(axis=0)
    mx = y.max(axis=0)
    step = (mx.astype(np.float64) - mn) / (P - 1)

    srt0 = np.sort(y[s_np == 0], axis=0)
    srt1 = np.sort(y[s_np == 1], axis=0)
    n0, n1 = srt0.shape[0], srt1.shape[0]

    v0, w0 = _rank_merge(srt0, TAU / TEMP)
    v1, w1 = _rank_merge(srt1, TAU / TEMP)

    jj = np.arange(W, dtype=np.float64)

    in_maps = []
    b_tabs = []
    core_meta = []
    for r in range(NCORES):
        vs, ws, gs = [], [], []
        for gi, (v, w) in enumerate(((v0, w0), (v1, w1))):
            idx = np.array_split(np.arange(len(w)), NCORES)[r]
            vs.append(v[idx])
            ws.append(w[idx])
            gs.append(np.full(len(idx), gi))
        vals = np.concatenate(vs)          # [cnt, C]
        wts = np.concatenate(ws)           # [cnt]
        grp = np.concatenate(gs)           # [cnt]
        cnt = len(wts)
        assert cnt <= PART, cnt

        ymax_t = vals.max(0).astype(np.float64)
        ymin_t = vals.min(0).astype(np.float64)
        needW = ((ymax_t - ymin_t + 2 * MARGIN) / step + 2).max()
        assert needW <= W, (needW, W)
        B = np.ceil((ymax_t + MARGIN - mn) / step).astype(np.int64) - W + 1
        B = np.clip(B, 0, P - W)
        base = mn.astype(np.float64) + step * B            # [C]

        # centered window args: diff = A'[m,c] + Dw[c,w]
        A = np.zeros((PART, C), np.float64)
        A[:cnt] = base[None, :] + step * (W / 2) - vals
        A[cnt:] = A[cnt - 1]                                # benign pad
        Dw = step[:, None] * (jj[None, :] - W / 2)          # [C, W]

        eg = np.zeros((K1, CW), np.float16)
        for c in range(C):
            eg[c, c * W : (c + 1) * W] = 1.0                # selector
        eg[C, :] = Dw.reshape(CW).astype(np.float16)        # Dw row
        c0_ = _CHUNKS[0][1]
        a = np.zeros((K1, PART + c0_), np.float16)
        a[0:C, 0:PART] = A.T.astype(np.float16)             # aug rows: A'
        a[C, 0:PART] = 1.0                                  # ones row
        a[:, PART:] = eg[:, 0:c0_]
        mask = np.zeros((PART, 2), np.float16)
        mask[np.arange(cnt), grp] = wts

        in_maps.append({"a": a, "a2": eg[:, c0_:].copy(), "m": mask})
        b_tabs.append(B)
        core_meta.append(mask.sum(0).astype(np.float64))    # [2] group weight

    nc = _get_nc()
    res = run_bass_kernel_spmd(
        nc,
        in_maps,
        core_ids=list(range(NCORES)),
        trace=bool(int(os.environ.get("BASS_KERNEL_TRACE", "0"))),
    )
    LAST_RESULTS = res

    full = np.zeros((2, C, P + W), np.float64)
    for r in range(NCORES):
        o = res.results[r]["o"].astype(np.float64)          # [2, CW]
        acc = o.reshape(2, C, W)
        B = b_tabs[r]
        gw = core_meta[r]
        for c in range(C):
            b = B[c]
            full[:, c, b : b + W] += acc[:, c, :]
            full[0, c, b + W :] += gw[0]
            full[1, c, b + W :] += gw[1]
    fullP = full[:, :, :P]
    delta = np.abs(fullP[0] / n0 - fullP[1] / n1)
    LAST_DELTA = delta
    return np.array(delta.max(), dtype=np.float32)
